# revision 8
# baseline (speedup 1.0000x reference)
"""Multi-head attention (B=2, S=2048, D=512, H=8) on 8 trn2 NeuronCores.

Sharding: data-parallel over batch (2) x tensor-parallel over head-pairs (4).
Core c handles batch c//4 and heads [2*(c%4), 2*(c%4)+1] (128 model dims).

Device kernel (SPMD, identical program, per-core inputs):
  inputs:  xqT/xkT/xvT [512,2048] (host-pretransposed), wq/wk/wv [512,128]
           (column slice), wo [128,512] (row slice), bq/bk [128,1]
  outputs: attn_out [2,2048,2048] (this core's two heads, softmaxed),
           out_partial [2048,512] (this core's contribution to out)

Host folds the v/o biases in afterwards: since each softmax row sums to 1,
ctx = attn@(vh + 1*bv^T) = attn@vh + bv, so out += bv @ wo_w + wo_b.
"""

import numpy as np

import concourse.bass as bass
import concourse.mybir as mybir
from concourse import bacc
from concourse.tile import TileContext
from concourse.bass_utils import run_bass_kernel_spmd

B, S, D = 2, 2048, 512
H, DEP = 8, 64
N_CORES = 8
HPC = 2          # heads per core
D2 = HPC * DEP   # 128 model dims per core
NT = S // 128    # 16 tiles of 128 rows
F32 = mybir.dt.float32
F16 = mybir.dt.float16

_CACHED_NC = None


def _build_nc():
    nc = bacc.Bacc(None, target_bir_lowering=False, debug=False)

    xqT = nc.declare_dram_parameter("xqT", [D, S], F32, isOutput=False)
    xkT = nc.declare_dram_parameter("xkT", [D, S], F32, isOutput=False)
    xvT = nc.declare_dram_parameter("xvT", [D, S], F32, isOutput=False)
    wq = nc.declare_dram_parameter("wq", [D, D2], F32, isOutput=False)
    wk = nc.declare_dram_parameter("wk", [D, D2], F32, isOutput=False)
    wv = nc.declare_dram_parameter("wv", [D, D2], F32, isOutput=False)
    wo = nc.declare_dram_parameter("wo", [D2, D], F32, isOutput=False)
    bq = nc.declare_dram_parameter("bq", [D2, 1], F32, isOutput=False)
    bk = nc.declare_dram_parameter("bk", [D2, 1], F32, isOutput=False)
    attn_out = nc.declare_dram_parameter("attn_out", [HPC, S, S], F32, isOutput=True)
    out_partial = nc.declare_dram_parameter("out_partial", [S, D], F32, isOutput=True)

    scale = 1.0 / np.sqrt(DEP)

    with TileContext(nc) as tc:
        with tc.tile_pool(name="singles", bufs=1) as singles:
            # persistent SBUF tensors
            qhT = singles.tile([D2, S], F16, tag="qhT")     # [128, 2048]
            khT = singles.tile([D2, S], F16, tag="khT")
            vh = singles.tile([128, S], F16, tag="vh")     # block st: [128(k),128(d)]
            ctxT = singles.tile([D2, S], F32, tag="ctxT")   # [128(d), 2048(q)] unnormalized
            recip = singles.tile([128, HPC * NT], F32, tag="recip")  # col h*16+qt
            wq_sb = singles.tile([128, 4, D2], F32, tag="wq")
            wk_sb = singles.tile([128, 4, D2], F32, tag="wk")
            wv_sb = singles.tile([128, 4, D2], F32, tag="wv")
            wo_sb = singles.tile([D2, D], F32, tag="wo")
            bq_sb = singles.tile([D2, 1], F32, tag="bq")
            bk_sb = singles.tile([D2, 1], F32, tag="bk")

            nc.sync.dma_start(out=wq_sb[:], in_=wq.ap().rearrange("(c p) d -> p c d", p=128))
            nc.sync.dma_start(out=wk_sb[:], in_=wk.ap().rearrange("(c p) d -> p c d", p=128))
            nc.sync.dma_start(out=wv_sb[:], in_=wv.ap().rearrange("(c p) d -> p c d", p=128))
            nc.sync.dma_start(out=wo_sb[:], in_=wo.ap())
            nc.sync.dma_start(out=bq_sb[:], in_=bq.ap())
            nc.sync.dma_start(out=bk_sb[:], in_=bk.ap())

            # ---------------- stage 1: projections ----------------
            HW = S // 2  # 1024-col halves of the sequence
            with (
                tc.tile_pool(name="xpool", bufs=2) as xpool,
                tc.tile_pool(name="psum1", bufs=2, space="PSUM") as psum1,
            ):
                # qhT = wq.T @ xqT (+bq), khT likewise
                for name, xT, w_sb, b_sb, outT in (
                    ("q", xqT, wq_sb, bq_sb, qhT),
                    ("k", xkT, wk_sb, bk_sb, khT),
                ):
                    for half in range(2):
                        x_sb = xpool.tile([128, 4, HW], F32, tag="x")
                        nc.sync.dma_start(
                            out=x_sb[:],
                            in_=xT.ap().rearrange("(c p) s -> p c s", p=128)[
                                :, :, half * HW : (half + 1) * HW
                            ],
                        )
                        ps = psum1.tile([D2, HW], F32, tag="ps1")
                        for ncx in range(HW // 512):
                            for cc in range(4):
                                nc.tensor.matmul(
                                    ps[:, ncx * 512 : (ncx + 1) * 512],
                                    w_sb[:, cc, :],
                                    x_sb[:, cc, ncx * 512 : (ncx + 1) * 512],
                                    start=(cc == 0),
                                    stop=(cc == 3),
                                )
                        nc.vector.tensor_scalar_add(
                            outT[:, half * HW : (half + 1) * HW], ps[:], b_sb[:]
                        )
                # vh[st block] = (xv @ wv) rows; bias folded on host
                for half in range(2):
                    x_sb = xpool.tile([128, 4, HW], F32, tag="x")
                    nc.sync.dma_start(
                        out=x_sb[:],
                        in_=xvT.ap().rearrange("(c p) s -> p c s", p=128)[
                            :, :, half * HW : (half + 1) * HW
                        ],
                    )
                    ps = psum1.tile([128, HW], F32, tag="ps1")
                    for sti in range(HW // 128):
                        for cc in range(4):
                            nc.tensor.matmul(
                                ps[:, sti * 128 : (sti + 1) * 128],
                                x_sb[:, cc, sti * 128 : (sti + 1) * 128],
                                wv_sb[:, cc, :],
                                start=(cc == 0),
                                stop=(cc == 3),
                            )
                    nc.vector.tensor_copy(
                        out=vh[:, half * HW : (half + 1) * HW], in_=ps[:]
                    )

            # ---------------- stage 2: attention ----------------
            with (
                tc.tile_pool(name="psum_p", bufs=1, space="PSUM") as pool_p,
                tc.tile_pool(name="psum_pt", bufs=1, space="PSUM") as pool_pt,
                tc.tile_pool(name="psum_av", bufs=2, space="PSUM") as pool_av,
                tc.tile_pool(name="Ppool", bufs=3) as Ppool,
                tc.tile_pool(name="PTpool", bufs=1) as PTpool,
                tc.tile_pool(name="sumpool", bufs=4) as sumpool,
            ):
                # Two phases: in phase p, run head hP's P-stream (attn rows,
                # partitions hP*64..) concurrently with head hT=1-hP's
                # PT-stream (transposed probs, the other PE row-group),
                # then a dense AV burst for hT.
                for phase in range(2):
                    hP, hT = phase, 1 - phase
                    hsP = slice(hP * DEP, (hP + 1) * DEP)
                    hsT = slice(hT * DEP, (hT + 1) * DEP)
                    PT = PTpool.tile([128, NT, S], F16, tag="PT")
                    for t in range(NT):
                        # ---- P stream (head hP, q-tile t) ----
                        ps_p = pool_p.tile([128, S], F32, tag="pp")
                        for ncx in range(4):
                            nc.tensor.matmul(
                                ps_p[:, ncx * 512 : (ncx + 1) * 512],
                                qhT[hsP, t * 128 : (t + 1) * 128],
                                khT[hsP, ncx * 512 : (ncx + 1) * 512],
                                start=True,
                                stop=True,
                            )
                        # ---- PT stream (head hT, k-tile t), interleaved ----
                        for half in range(2):
                            ps_t = pool_pt.tile([128, 1024], F32, tag="pt")
                            for ncx in range(2):
                                nc.tensor.matmul(
                                    ps_t[:, ncx * 512 : (ncx + 1) * 512],
                                    khT[hsT, t * 128 : (t + 1) * 128],
                                    qhT[hsT, half * 1024 + ncx * 512 : half * 1024 + (ncx + 1) * 512],
                                    start=True,
                                    stop=True,
                                )
                            nc.scalar.activation(
                                PT[:, t, half * 1024 : (half + 1) * 1024],
                                ps_t[:],
                                mybir.ActivationFunctionType.Exp,
                                scale=float(scale),
                            )
                        P_sb = Ppool.tile([128, S], F32, tag="P")
                        sums = sumpool.tile([128, 1], F32, tag="sums")
                        nc.scalar.activation(
                            P_sb[:],
                            ps_p[:],
                            mybir.ActivationFunctionType.Exp,
                            scale=float(scale),
                            accum_out=sums[:],
                        )
                        rc = recip[:, hP * NT + t : hP * NT + t + 1]
                        nc.vector.reciprocal(rc, sums[:])
                        nc.vector.tensor_scalar_mul(P_sb[:], P_sb[:], rc)
                        nc.sync.dma_start(
                            out=attn_out[hP, t * 128 : (t + 1) * 128, :], in_=P_sb[:]
                        )
                    # ---- AV burst: ctxT[hT] = sum_kt vh_hT[kt].T @ PT[kt] ----
                    for qc in range(4):
                        ps_av = pool_av.tile([DEP, 512], F32, tag="av")
                        for kt in range(NT):
                            nc.tensor.matmul(
                                ps_av[:],
                                vh[:, kt * 128 + hT * DEP : kt * 128 + (hT + 1) * DEP],
                                PT[:, kt, qc * 512 : (qc + 1) * 512],
                                start=(kt == 0),
                                stop=(kt == NT - 1),
                            )
                        nc.vector.tensor_copy(
                            out=ctxT[hsT, qc * 512 : (qc + 1) * 512], in_=ps_av[:]
                        )

            # ---------------- stage 3: output projection ----------------
            with (
                tc.tile_pool(name="psum_o", bufs=4, space="PSUM") as pool_o,
                tc.tile_pool(name="opool", bufs=3) as opool,
            ):
                for t in range(NT):
                    acc = None
                    for h in range(HPC):
                        hs = slice(h * DEP, (h + 1) * DEP)
                        ps_o = pool_o.tile([128, D], F32, tag="po")
                        nc.tensor.matmul(
                            ps_o[:],
                            ctxT[hs, t * 128 : (t + 1) * 128],
                            wo_sb[hs, :],
                            start=True,
                            stop=True,
                        )
                        tmp = opool.tile([128, D], F32, tag=f"otmp{h}")
                        nc.vector.tensor_scalar_mul(
                            tmp[:], ps_o[:], recip[:, h * NT + t : h * NT + t + 1]
                        )
                        if acc is None:
                            acc = tmp
                        else:
                            nc.vector.tensor_add(acc[:], acc[:], tmp[:])
                    nc.sync.dma_start(
                        out=out_partial[t * 128 : (t + 1) * 128, :], in_=acc[:]
                    )

    nc.finalize()
    return nc


def kernel(q, k, v, wq_w, wq_b, wk_w, wk_b, wv_w, wv_b, wo_w, wo_b, _profile=False):
    global _CACHED_NC
    q = np.asarray(q, np.float32)
    k = np.asarray(k, np.float32)
    v = np.asarray(v, np.float32)
    wq_w = np.asarray(wq_w, np.float32)
    wk_w = np.asarray(wk_w, np.float32)
    wv_w = np.asarray(wv_w, np.float32)
    wo_w = np.asarray(wo_w, np.float32)

    if _CACHED_NC is None:
        _CACHED_NC = _build_nc()
    nc = _CACHED_NC

    xT = {}
    for b in range(B):
        xT[("q", b)] = np.ascontiguousarray(q[b].T)
        xT[("k", b)] = np.ascontiguousarray(k[b].T)
        xT[("v", b)] = np.ascontiguousarray(v[b].T)

    in_maps = []
    for c in range(N_CORES):
        b, hp = divmod(c, 4)
        cs = slice(hp * D2, (hp + 1) * D2)
        in_maps.append(
            {
                "xqT": xT[("q", b)],
                "xkT": xT[("k", b)],
                "xvT": xT[("v", b)],
                "wq": np.ascontiguousarray(wq_w[:, cs]),
                "wk": np.ascontiguousarray(wk_w[:, cs]),
                "wv": np.ascontiguousarray(wv_w[:, cs]),
                "wo": np.ascontiguousarray(wo_w[cs, :]),
                "bq": np.ascontiguousarray(np.asarray(wq_b, np.float32)[cs, None]),
                "bk": np.ascontiguousarray(np.asarray(wk_b, np.float32)[cs, None]),
            }
        )

    kwargs = {}
    if _profile:
        import os

        os.makedirs("/tmp/bass_trace", exist_ok=True)
        kwargs = {"trace": True, "tmpdir": "/tmp/bass_trace"}
    res = run_bass_kernel_spmd(nc, in_maps, list(range(N_CORES)), **kwargs)

    attn = np.empty((B, H, S, S), np.float32)
    out = np.zeros((B, S, D), np.float32)
    for c in range(N_CORES):
        b, hp = divmod(c, 4)
        attn[b, 2 * hp : 2 * hp + 2] = res.results[c]["attn_out"]
        out[b] += res.results[c]["out_partial"]
    # fold v/o biases: softmax rows sum to 1 -> ctx += wv_b, out += wv_b@wo + wo_b
    out += (
        np.asarray(wv_b, np.float32) @ wo_w + np.asarray(wo_b, np.float32)
    )[None, None, :]

    if _profile:
        return (out, attn), res
    return out, attn


# revision 11
# speedup vs baseline: 1.2068x; 1.2068x over previous
"""Multi-head attention (B=2, S=2048, D=512, H=8) on 8 trn2 NeuronCores.

Sharding: data-parallel over batch (2) x tensor-parallel over head-pairs (4).
Core c handles batch c//4 and heads [2*(c%4), 2*(c%4)+1] (128 model dims).

Device kernel (SPMD, identical program, per-core inputs):
  inputs:  xqT/xkT/xvT [512,2048] (host-pretransposed), wq/wk/wv [512,128]
           (column slice), wo [128,512] (row slice), bq/bk [128,1]
  outputs: attn_out [2,2048,2048] (this core's two heads, softmaxed),
           out_partial [2048,512] (this core's contribution to out)

Host folds the v/o biases in afterwards: since each softmax row sums to 1,
ctx = attn@(vh + 1*bv^T) = attn@vh + bv, so out += bv @ wo_w + wo_b.
"""

import numpy as np

import concourse.bass as bass
import concourse.mybir as mybir
from concourse import bacc
from concourse.tile import TileContext
from concourse.bass_utils import run_bass_kernel_spmd

B, S, D = 2, 2048, 512
H, DEP = 8, 64
N_CORES = 8
HPC = 2          # heads per core
D2 = HPC * DEP   # 128 model dims per core
NT = S // 128    # 16 tiles of 128 rows
F32 = mybir.dt.float32
F16 = mybir.dt.float16

_CACHED_NC = None


def _build_nc():
    nc = bacc.Bacc(None, target_bir_lowering=False, debug=False)

    xqT = nc.declare_dram_parameter("xqT", [D, S], F32, isOutput=False)
    xkT = nc.declare_dram_parameter("xkT", [D, S], F32, isOutput=False)
    xvT = nc.declare_dram_parameter("xvT", [D, S], F32, isOutput=False)
    wq = nc.declare_dram_parameter("wq", [D, D2], F32, isOutput=False)
    wk = nc.declare_dram_parameter("wk", [D, D2], F32, isOutput=False)
    wv = nc.declare_dram_parameter("wv", [D, D2], F32, isOutput=False)
    wo = nc.declare_dram_parameter("wo", [D2, D], F32, isOutput=False)
    bq = nc.declare_dram_parameter("bq", [D2, 1], F32, isOutput=False)
    bk = nc.declare_dram_parameter("bk", [D2, 1], F32, isOutput=False)
    attn_out = nc.declare_dram_parameter("attn_out", [HPC, S, S], F32, isOutput=True)
    out_partial = nc.declare_dram_parameter("out_partial", [S, D], F32, isOutput=True)

    scale = 1.0 / np.sqrt(DEP)

    with TileContext(nc) as tc:
        with tc.tile_pool(name="singles", bufs=1) as singles:
            # persistent SBUF tensors
            qhT = singles.tile([D2, S], F16, tag="qhT")     # [128, 2048]
            khT = singles.tile([D2, S], F16, tag="khT")
            vh = singles.tile([128, S], F16, tag="vh")     # block st: [128(k),128(d)]
            ctxT = singles.tile([D2, S], F32, tag="ctxT")   # [128(d), 2048(q)] unnormalized
            recip = singles.tile([128, HPC * NT], F32, tag="recip")  # col h*16+qt
            wq_sb = singles.tile([128, 4, D2], F32, tag="wq")
            wk_sb = singles.tile([128, 4, D2], F32, tag="wk")
            wv_sb = singles.tile([128, 4, D2], F32, tag="wv")
            wo_sb = singles.tile([D2, D], F32, tag="wo")
            bq_sb = singles.tile([D2, 1], F32, tag="bq")
            bk_sb = singles.tile([D2, 1], F32, tag="bk")

            nc.sync.dma_start(out=wq_sb[:], in_=wq.ap().rearrange("(c p) d -> p c d", p=128))
            nc.sync.dma_start(out=wk_sb[:], in_=wk.ap().rearrange("(c p) d -> p c d", p=128))
            nc.sync.dma_start(out=wv_sb[:], in_=wv.ap().rearrange("(c p) d -> p c d", p=128))
            nc.sync.dma_start(out=wo_sb[:], in_=wo.ap())
            nc.sync.dma_start(out=bq_sb[:], in_=bq.ap())
            nc.sync.dma_start(out=bk_sb[:], in_=bk.ap())

            # ---------------- stage 1: projections ----------------
            HW = S // 2  # 1024-col halves of the sequence
            with (
                tc.tile_pool(name="xpool", bufs=2) as xpool,
                tc.tile_pool(name="psum1", bufs=2, space="PSUM") as psum1,
            ):
                # qhT = wq.T @ xqT (+bq), khT likewise
                for name, xT, w_sb, b_sb, outT in (
                    ("q", xqT, wq_sb, bq_sb, qhT),
                    ("k", xkT, wk_sb, bk_sb, khT),
                ):
                    for half in range(2):
                        x_sb = xpool.tile([128, 4, HW], F32, tag="x")
                        nc.sync.dma_start(
                            out=x_sb[:],
                            in_=xT.ap().rearrange("(c p) s -> p c s", p=128)[
                                :, :, half * HW : (half + 1) * HW
                            ],
                        )
                        ps = psum1.tile([D2, HW], F32, tag="ps1")
                        for ncx in range(HW // 512):
                            for cc in range(4):
                                nc.tensor.matmul(
                                    ps[:, ncx * 512 : (ncx + 1) * 512],
                                    w_sb[:, cc, :],
                                    x_sb[:, cc, ncx * 512 : (ncx + 1) * 512],
                                    start=(cc == 0),
                                    stop=(cc == 3),
                                )
                        nc.vector.tensor_scalar_add(
                            outT[:, half * HW : (half + 1) * HW], ps[:], b_sb[:]
                        )
                # vh[st block] = (xv @ wv) rows; bias folded on host
                for half in range(2):
                    x_sb = xpool.tile([128, 4, HW], F32, tag="x")
                    nc.sync.dma_start(
                        out=x_sb[:],
                        in_=xvT.ap().rearrange("(c p) s -> p c s", p=128)[
                            :, :, half * HW : (half + 1) * HW
                        ],
                    )
                    ps = psum1.tile([128, HW], F32, tag="ps1")
                    for sti in range(HW // 128):
                        for cc in range(4):
                            nc.tensor.matmul(
                                ps[:, sti * 128 : (sti + 1) * 128],
                                x_sb[:, cc, sti * 128 : (sti + 1) * 128],
                                wv_sb[:, cc, :],
                                start=(cc == 0),
                                stop=(cc == 3),
                            )
                    nc.vector.tensor_copy(
                        out=vh[:, half * HW : (half + 1) * HW], in_=ps[:]
                    )

            # ---------------- stage 2: attention ----------------
            with (
                tc.tile_pool(name="psum_p", bufs=1, space="PSUM") as pool_p,
                tc.tile_pool(name="psum_pt", bufs=2, space="PSUM") as pool_pt,
                tc.tile_pool(name="Ppool", bufs=3) as Ppool,
                tc.tile_pool(name="PTpool", bufs=1) as PTpool,
                tc.tile_pool(name="sumpool", bufs=4) as sumpool,
            ):
                # Two phases: in phase p, run head hP's P-stream (attn rows,
                # partitions hP*64..) concurrently with head hT=1-hP's
                # PT-stream (transposed probs, the other PE row-group),
                # then a dense AV burst for hT.
                for phase in range(2):
                    hP, hT = phase, 1 - phase
                    hsP = slice(hP * DEP, (hP + 1) * DEP)
                    hsT = slice(hT * DEP, (hT + 1) * DEP)
                    PT = PTpool.tile([128, NT, S], F16, tag="PT")
                    for t in range(NT):
                        # ---- PT stream (head hT, k-tile t) ----
                        for half in range(2):
                            ps_t = pool_pt.tile([128, 1024], F32, tag="pt")
                            for ncx in range(2):
                                nc.tensor.matmul(
                                    ps_t[:, ncx * 512 : (ncx + 1) * 512],
                                    khT[hsT, t * 128 : (t + 1) * 128],
                                    qhT[hsT, half * 1024 + ncx * 512 : half * 1024 + (ncx + 1) * 512],
                                    start=True,
                                    stop=True,
                                )
                            nc.scalar.activation(
                                PT[:, t, half * 1024 : (half + 1) * 1024],
                                ps_t[:],
                                mybir.ActivationFunctionType.Exp,
                                scale=float(scale),
                            )
                        # ---- P stream (head hP, q-tile t), other row-group ----
                        ps_p = pool_p.tile([128, S], F32, tag="pp")
                        for ncx in range(4):
                            nc.tensor.matmul(
                                ps_p[:, ncx * 512 : (ncx + 1) * 512],
                                qhT[hsP, t * 128 : (t + 1) * 128],
                                khT[hsP, ncx * 512 : (ncx + 1) * 512],
                                start=True,
                                stop=True,
                            )
                        P_sb = Ppool.tile([128, S], F32, tag="P")
                        sums = sumpool.tile([128, 1], F32, tag="sums")
                        nc.scalar.activation(
                            P_sb[:],
                            ps_p[:],
                            mybir.ActivationFunctionType.Exp,
                            scale=float(scale),
                            accum_out=sums[:],
                        )
                        rc = recip[:, hP * NT + t : hP * NT + t + 1]
                        nc.vector.reciprocal(rc, sums[:])
                        nc.vector.tensor_scalar_mul(P_sb[:], P_sb[:], rc)
                        nc.sync.dma_start(
                            out=attn_out[hP, t * 128 : (t + 1) * 128, :], in_=P_sb[:]
                        )
                    # ---- AV burst: ctxT[hT] = sum_kt vh_hT[kt].T @ PT[kt] ----
                    # two 8-deep chains through the pt slots, then one DVE add
                    for qc in range(4):
                        avs = []
                        for c in range(2):
                            ps_av = pool_pt.tile([DEP, 512], F32, tag="pt")
                            for j in range(8):
                                kt = c * 8 + j
                                nc.tensor.matmul(
                                    ps_av[:],
                                    vh[:, kt * 128 + hT * DEP : kt * 128 + (hT + 1) * DEP],
                                    PT[:, kt, qc * 512 : (qc + 1) * 512],
                                    start=(j == 0),
                                    stop=(j == 7),
                                )
                            avs.append(ps_av)
                        ct = ctxT[hsT, qc * 512 : (qc + 1) * 512]
                        nc.vector.tensor_copy(out=ct, in_=avs[0][:])
                        nc.vector.tensor_add(ct, ct, avs[1][:])

            # ---------------- stage 3: output projection ----------------
            with (
                tc.tile_pool(name="psum_o", bufs=4, space="PSUM") as pool_o,
                tc.tile_pool(name="opool", bufs=3) as opool,
            ):
                for t in range(NT):
                    acc = None
                    for h in range(HPC):
                        hs = slice(h * DEP, (h + 1) * DEP)
                        ps_o = pool_o.tile([128, D], F32, tag="po")
                        nc.tensor.matmul(
                            ps_o[:],
                            ctxT[hs, t * 128 : (t + 1) * 128],
                            wo_sb[hs, :],
                            start=True,
                            stop=True,
                        )
                        tmp = opool.tile([128, D], F32, tag=f"otmp{h}")
                        nc.vector.tensor_scalar_mul(
                            tmp[:], ps_o[:], recip[:, h * NT + t : h * NT + t + 1]
                        )
                        if acc is None:
                            acc = tmp
                        else:
                            nc.vector.tensor_add(acc[:], acc[:], tmp[:])
                    nc.sync.dma_start(
                        out=out_partial[t * 128 : (t + 1) * 128, :], in_=acc[:]
                    )

    nc.finalize()
    return nc


def kernel(q, k, v, wq_w, wq_b, wk_w, wk_b, wv_w, wv_b, wo_w, wo_b, _profile=False):
    global _CACHED_NC
    q = np.asarray(q, np.float32)
    k = np.asarray(k, np.float32)
    v = np.asarray(v, np.float32)
    wq_w = np.asarray(wq_w, np.float32)
    wk_w = np.asarray(wk_w, np.float32)
    wv_w = np.asarray(wv_w, np.float32)
    wo_w = np.asarray(wo_w, np.float32)

    if _CACHED_NC is None:
        _CACHED_NC = _build_nc()
    nc = _CACHED_NC

    xT = {}
    for b in range(B):
        xT[("q", b)] = np.ascontiguousarray(q[b].T)
        xT[("k", b)] = np.ascontiguousarray(k[b].T)
        xT[("v", b)] = np.ascontiguousarray(v[b].T)

    in_maps = []
    for c in range(N_CORES):
        b, hp = divmod(c, 4)
        cs = slice(hp * D2, (hp + 1) * D2)
        in_maps.append(
            {
                "xqT": xT[("q", b)],
                "xkT": xT[("k", b)],
                "xvT": xT[("v", b)],
                "wq": np.ascontiguousarray(wq_w[:, cs]),
                "wk": np.ascontiguousarray(wk_w[:, cs]),
                "wv": np.ascontiguousarray(wv_w[:, cs]),
                "wo": np.ascontiguousarray(wo_w[cs, :]),
                "bq": np.ascontiguousarray(np.asarray(wq_b, np.float32)[cs, None]),
                "bk": np.ascontiguousarray(np.asarray(wk_b, np.float32)[cs, None]),
            }
        )

    kwargs = {}
    if _profile:
        import os

        os.makedirs("/tmp/bass_trace", exist_ok=True)
        kwargs = {"trace": True, "tmpdir": "/tmp/bass_trace"}
    res = run_bass_kernel_spmd(nc, in_maps, list(range(N_CORES)), **kwargs)

    attn = np.empty((B, H, S, S), np.float32)
    out = np.zeros((B, S, D), np.float32)
    for c in range(N_CORES):
        b, hp = divmod(c, 4)
        attn[b, 2 * hp : 2 * hp + 2] = res.results[c]["attn_out"]
        out[b] += res.results[c]["out_partial"]
    # fold v/o biases: softmax rows sum to 1 -> ctx += wv_b, out += wv_b@wo + wo_b
    out += (
        np.asarray(wv_b, np.float32) @ wo_w + np.asarray(wo_b, np.float32)
    )[None, None, :]

    if _profile:
        return (out, attn), res
    return out, attn


# revision 20
# speedup vs baseline: 1.3129x; 1.0879x over previous
"""Multi-head attention (B=2, S=2048, D=512, H=8) on 8 trn2 NeuronCores.

Sharding: data-parallel over batch (2) x tensor-parallel over head-pairs (4).
Core c handles batch c//4 and heads [2*(c%4), 2*(c%4)+1] (128 model dims).

Device kernel (SPMD, identical program, per-core inputs):
  inputs:  xqT/xkT/xvT [512,2048] (host-pretransposed), wq/wk/wv [512,128]
           (column slice), wo [128,512] (row slice), bq/bk [128,1]
  outputs: attn_out [2,2048,2048] (this core's two heads, softmaxed),
           out_partial [2048,512] (this core's contribution to out)

Host folds the v/o biases in afterwards: since each softmax row sums to 1,
ctx = attn@(vh + 1*bv^T) = attn@vh + bv, so out += bv @ wo_w + wo_b.
"""

import numpy as np

import concourse.bass as bass
import concourse.mybir as mybir
from concourse import bacc
from concourse.tile import TileContext
from concourse.bass_utils import run_bass_kernel_spmd

B, S, D = 2, 2048, 512
H, DEP = 8, 64
N_CORES = 8
HPC = 2          # heads per core
D2 = HPC * DEP   # 128 model dims per core
NT = S // 128    # 16 tiles of 128 rows
F32 = mybir.dt.float32
F16 = mybir.dt.float16

_CACHED_NC = None


def _build_nc():
    nc = bacc.Bacc(None, target_bir_lowering=False, debug=False)

    xqT = nc.declare_dram_parameter("xqT", [D, S], F16, isOutput=False)
    xkT = nc.declare_dram_parameter("xkT", [D, S], F16, isOutput=False)
    xvT = nc.declare_dram_parameter("xvT", [D, S], F16, isOutput=False)
    wq = nc.declare_dram_parameter("wq", [D, D2], F16, isOutput=False)
    wk = nc.declare_dram_parameter("wk", [D, D2], F16, isOutput=False)
    wv = nc.declare_dram_parameter("wv", [D, D2], F16, isOutput=False)
    wo = nc.declare_dram_parameter("wo", [D2, D], F16, isOutput=False)
    bq = nc.declare_dram_parameter("bq", [D2, 1], F32, isOutput=False)
    bk = nc.declare_dram_parameter("bk", [D2, 1], F32, isOutput=False)
    attn_out = nc.declare_dram_parameter("attn_out", [HPC, S, S], F32, isOutput=True)
    out_partial = nc.declare_dram_parameter("out_partial", [S, D], F32, isOutput=True)

    scale = 1.0 / np.sqrt(DEP)

    with TileContext(nc) as tc:
        with tc.tile_pool(name="singles", bufs=1) as singles:
            # persistent SBUF tensors
            qhT = singles.tile([D2, S], F16, tag="qhT")     # [128, 2048]
            khT = singles.tile([D2, S], F16, tag="khT")
            vh = singles.tile([128, S], F16, tag="vh")     # block st: [128(k),128(d)]
            ctxT = singles.tile([D2, S], F16, tag="ctxT")   # [128(d), 2048(q)] unnormalized
            recip = singles.tile([128, HPC * NT], F32, tag="recip")  # col h*16+qt
            wq_sb = singles.tile([128, 4, D2], F16, tag="wq")
            wk_sb = singles.tile([128, 4, D2], F16, tag="wk")
            wv_sb = singles.tile([128, 4, D2], F16, tag="wv")
            wo_sb = singles.tile([D2, D], F16, tag="wo")
            bq_sb = singles.tile([D2, 1], F32, tag="bq")
            bk_sb = singles.tile([D2, 1], F32, tag="bk")

            nc.sync.dma_start(out=wq_sb[:], in_=wq.ap().rearrange("(c p) d -> p c d", p=128))
            nc.sync.dma_start(out=wk_sb[:], in_=wk.ap().rearrange("(c p) d -> p c d", p=128))
            nc.sync.dma_start(out=wv_sb[:], in_=wv.ap().rearrange("(c p) d -> p c d", p=128))
            nc.sync.dma_start(out=wo_sb[:], in_=wo.ap())
            nc.sync.dma_start(out=bq_sb[:], in_=bq.ap())
            nc.sync.dma_start(out=bk_sb[:], in_=bk.ap())

            # ---------------- stage 1: projections (all fp16 inputs) -------
            with (
                tc.tile_pool(name="xpool", bufs=2) as xpool,
                tc.tile_pool(name="psum1", bufs=2, space="PSUM") as psum1,
            ):
                # v first: vh is not needed until the first AV burst, and
                # q/k gate the main loop.
                x_v = xpool.tile([128, 4, S], F16, tag="x")
                nc.sync.dma_start(
                    out=x_v[:], in_=xvT.ap().rearrange("(c p) s -> p c s", p=128)
                )
                ps = psum1.tile([128, S], F32, tag="ps1")
                for sti in range(NT):
                    for cc in range(4):
                        nc.tensor.matmul(
                            ps[:, sti * 128 : (sti + 1) * 128],
                            x_v[:, cc, sti * 128 : (sti + 1) * 128],
                            wv_sb[:, cc, :],
                            start=(cc == 0),
                            stop=(cc == 3),
                        )
                nc.vector.tensor_copy(out=vh[:], in_=ps[:])
                # qhT = wq.T @ xqT (+bq), khT likewise
                for xT, w_sb, b_sb, outT in (
                    (xqT, wq_sb, bq_sb, qhT),
                    (xkT, wk_sb, bk_sb, khT),
                ):
                    x_sb = xpool.tile([128, 4, S], F16, tag="x")
                    nc.sync.dma_start(
                        out=x_sb[:], in_=xT.ap().rearrange("(c p) s -> p c s", p=128)
                    )
                    ps = psum1.tile([D2, S], F32, tag="ps1")
                    for ncx in range(4):
                        for cc in range(4):
                            nc.tensor.matmul(
                                ps[:, ncx * 512 : (ncx + 1) * 512],
                                w_sb[:, cc, :],
                                x_sb[:, cc, ncx * 512 : (ncx + 1) * 512],
                                start=(cc == 0),
                                stop=(cc == 3),
                            )
                    nc.vector.tensor_scalar_add(outT[:], ps[:], b_sb[:])

            # ---------------- stage 2: attention ----------------
            with (
                tc.tile_pool(name="psum_p", bufs=1, space="PSUM") as pool_p,
                tc.tile_pool(name="psum_pt", bufs=2, space="PSUM") as pool_pt,
                tc.tile_pool(name="Ppool", bufs=3) as Ppool,
                tc.tile_pool(name="PTpool", bufs=1) as PTpool,
                tc.tile_pool(name="sumpool", bufs=4) as sumpool,
            ):
                # Two phases: in phase p, run head hP's P-stream (attn rows,
                # partitions hP*64..) concurrently with head hT=1-hP's
                # PT-stream (transposed probs, the other PE row-group),
                # then a dense AV burst for hT.
                for phase in range(2):
                    hP, hT = phase, 1 - phase
                    hsP = slice(hP * DEP, (hP + 1) * DEP)
                    hsT = slice(hT * DEP, (hT + 1) * DEP)
                    PT = PTpool.tile([128, NT, S], F16, tag="PT")
                    for t in range(NT):
                        # ---- PT stream (head hT, k-tile t) ----
                        for half in range(2):
                            ps_t = pool_pt.tile([128, 1024], F32, tag="pt")
                            for ncx in range(2):
                                nc.tensor.matmul(
                                    ps_t[:, ncx * 512 : (ncx + 1) * 512],
                                    khT[hsT, t * 128 : (t + 1) * 128],
                                    qhT[hsT, half * 1024 + ncx * 512 : half * 1024 + (ncx + 1) * 512],
                                    start=True,
                                    stop=True,
                                )
                            nc.scalar.activation(
                                PT[:, t, half * 1024 : (half + 1) * 1024],
                                ps_t[:],
                                mybir.ActivationFunctionType.Exp,
                                scale=float(scale),
                            )
                        # ---- P stream (head hP, q-tile t), other row-group ----
                        ps_p = pool_p.tile([128, S], F32, tag="pp")
                        for ncx in range(4):
                            nc.tensor.matmul(
                                ps_p[:, ncx * 512 : (ncx + 1) * 512],
                                qhT[hsP, t * 128 : (t + 1) * 128],
                                khT[hsP, ncx * 512 : (ncx + 1) * 512],
                                start=True,
                                stop=True,
                            )
                        P_sb = Ppool.tile([128, S], F32, tag="P")
                        sums = sumpool.tile([128, 1], F32, tag="sums")
                        nc.scalar.activation(
                            P_sb[:],
                            ps_p[:],
                            mybir.ActivationFunctionType.Exp,
                            scale=float(scale),
                            accum_out=sums[:],
                        )
                        rc = recip[:, hP * NT + t : hP * NT + t + 1]
                        nc.vector.reciprocal(rc, sums[:])
                        nc.vector.tensor_scalar_mul(P_sb[:], P_sb[:], rc)
                        nc.sync.dma_start(
                            out=attn_out[hP, t * 128 : (t + 1) * 128, :], in_=P_sb[:]
                        )
                    # ---- AV burst: ctxT[hT] = sum_kt vh_hT[kt].T @ PT[kt] ----
                    # two 8-deep chains through the pt slots, then one DVE add
                    for qc in range(4):
                        avs = []
                        for c in range(2):
                            ps_av = pool_pt.tile([DEP, 512], F32, tag="pt")
                            for j in range(8):
                                kt = c * 8 + j
                                nc.tensor.matmul(
                                    ps_av[:],
                                    vh[:, kt * 128 + hT * DEP : kt * 128 + (hT + 1) * DEP],
                                    PT[:, kt, qc * 512 : (qc + 1) * 512],
                                    start=(j == 0),
                                    stop=(j == 7),
                                )
                            avs.append(ps_av)
                        ct = ctxT[hsT, qc * 512 : (qc + 1) * 512]
                        nc.vector.tensor_copy(out=ct, in_=avs[0][:])
                        nc.vector.tensor_add(ct, ct, avs[1][:])

                # ---- tail: output projection, overlapped with AV-B ----
                # head-1 scaled on ACT (copy with per-partition scale),
                # head-0 scaled on DVE, summed, written out.
                with tc.tile_pool(name="opool", bufs=3) as opool:
                    ps_o = pool_p.tile([128, S], F32, tag="pp")
                    for t in range(NT):
                        s1 = ps_o[:, ((2 * t) % 4) * 512 : ((2 * t) % 4 + 1) * 512]
                        s0 = ps_o[:, ((2 * t + 1) % 4) * 512 : ((2 * t + 1) % 4 + 1) * 512]
                        nc.tensor.matmul(
                            s1,
                            ctxT[DEP:, t * 128 : (t + 1) * 128],
                            wo_sb[DEP:, :],
                            start=True,
                            stop=True,
                        )
                        nc.tensor.matmul(
                            s0,
                            ctxT[:DEP, t * 128 : (t + 1) * 128],
                            wo_sb[:DEP, :],
                            start=True,
                            stop=True,
                        )
                        o1 = opool.tile([128, D], F32, tag="o1")
                        nc.scalar.activation(
                            o1[:],
                            s1,
                            mybir.ActivationFunctionType.Copy,
                            scale=recip[:, NT + t : NT + t + 1],
                        )
                        acc = opool.tile([128, D], F32, tag="acc")
                        nc.vector.tensor_scalar_mul(
                            acc[:], s0, recip[:, t : t + 1]
                        )
                        nc.vector.tensor_add(acc[:], acc[:], o1[:])
                        nc.sync.dma_start(
                            out=out_partial[t * 128 : (t + 1) * 128, :], in_=acc[:]
                        )

    nc.finalize()
    return nc


def kernel(q, k, v, wq_w, wq_b, wk_w, wk_b, wv_w, wv_b, wo_w, wo_b, _profile=False):
    global _CACHED_NC
    q = np.asarray(q, np.float32)
    k = np.asarray(k, np.float32)
    v = np.asarray(v, np.float32)
    wq_w = np.asarray(wq_w, np.float32)
    wk_w = np.asarray(wk_w, np.float32)
    wv_w = np.asarray(wv_w, np.float32)
    wo_w = np.asarray(wo_w, np.float32)

    if _CACHED_NC is None:
        _CACHED_NC = _build_nc()
    nc = _CACHED_NC

    xT = {}
    for b in range(B):
        xT[("q", b)] = np.ascontiguousarray(q[b].T.astype(np.float16))
        xT[("k", b)] = np.ascontiguousarray(k[b].T.astype(np.float16))
        xT[("v", b)] = np.ascontiguousarray(v[b].T.astype(np.float16))

    in_maps = []
    for c in range(N_CORES):
        b, hp = divmod(c, 4)
        cs = slice(hp * D2, (hp + 1) * D2)
        in_maps.append(
            {
                "xqT": xT[("q", b)],
                "xkT": xT[("k", b)],
                "xvT": xT[("v", b)],
                "wq": np.ascontiguousarray(wq_w[:, cs].astype(np.float16)),
                "wk": np.ascontiguousarray(wk_w[:, cs].astype(np.float16)),
                "wv": np.ascontiguousarray(wv_w[:, cs].astype(np.float16)),
                "wo": np.ascontiguousarray(wo_w[cs, :].astype(np.float16)),
                "bq": np.ascontiguousarray(np.asarray(wq_b, np.float32)[cs, None]),
                "bk": np.ascontiguousarray(np.asarray(wk_b, np.float32)[cs, None]),
            }
        )

    kwargs = {}
    if _profile:
        import os

        os.makedirs("/tmp/bass_trace", exist_ok=True)
        kwargs = {"trace": True, "tmpdir": "/tmp/bass_trace"}
    res = run_bass_kernel_spmd(nc, in_maps, list(range(N_CORES)), **kwargs)

    attn = np.empty((B, H, S, S), np.float32)
    out = np.zeros((B, S, D), np.float32)
    for c in range(N_CORES):
        b, hp = divmod(c, 4)
        attn[b, 2 * hp : 2 * hp + 2] = res.results[c]["attn_out"]
        out[b] += res.results[c]["out_partial"]
    # fold v/o biases: softmax rows sum to 1 -> ctx += wv_b, out += wv_b@wo + wo_b
    out += (
        np.asarray(wv_b, np.float32) @ wo_w + np.asarray(wo_b, np.float32)
    )[None, None, :]

    if _profile:
        return (out, attn), res
    return out, attn


# revision 23
# speedup vs baseline: 1.3718x; 1.0448x over previous
"""Multi-head attention (B=2, S=2048, D=512, H=8) on 8 trn2 NeuronCores.

Sharding: data-parallel over batch (2) x tensor-parallel over head-pairs (4).
Core c handles batch c//4 and heads [2*(c%4), 2*(c%4)+1] (128 model dims).

Device kernel (SPMD, identical program, per-core inputs):
  inputs:  xqT/xkT/xvT [512,2048] (host-pretransposed), wq/wk/wv [512,128]
           (column slice), wo [128,512] (row slice), bq/bk [128,1]
  outputs: attn_out [2,2048,2048] (this core's two heads, softmaxed),
           out_partial [2048,512] (this core's contribution to out)

Host folds the v/o biases in afterwards: since each softmax row sums to 1,
ctx = attn@(vh + 1*bv^T) = attn@vh + bv, so out += bv @ wo_w + wo_b.
"""

import numpy as np

import concourse.bass as bass
import concourse.mybir as mybir
from concourse import bacc
from concourse.tile import TileContext
from concourse.bass_utils import run_bass_kernel_spmd

B, S, D = 2, 2048, 512
H, DEP = 8, 64
N_CORES = 8
HPC = 2          # heads per core
D2 = HPC * DEP   # 128 model dims per core
NT = S // 128    # 16 tiles of 128 rows
F32 = mybir.dt.float32
F16 = mybir.dt.float16

_CACHED_NC = None


def _build_nc():
    nc = bacc.Bacc(None, target_bir_lowering=False, debug=False)

    xqT = nc.declare_dram_parameter("xqT", [D, S], F16, isOutput=False)
    xkT = nc.declare_dram_parameter("xkT", [D, S], F16, isOutput=False)
    xvT = nc.declare_dram_parameter("xvT", [D, S], F16, isOutput=False)
    wq = nc.declare_dram_parameter("wq", [D, D2], F16, isOutput=False)
    wk = nc.declare_dram_parameter("wk", [D, D2], F16, isOutput=False)
    wv = nc.declare_dram_parameter("wv", [D, D2], F16, isOutput=False)
    wo = nc.declare_dram_parameter("wo", [D2, D], F16, isOutput=False)
    bq = nc.declare_dram_parameter("bq", [D2, 1], F32, isOutput=False)
    bk = nc.declare_dram_parameter("bk", [D2, 1], F32, isOutput=False)
    attn_out = nc.declare_dram_parameter("attn_out", [HPC, S, S], F32, isOutput=True)
    out_partial = nc.declare_dram_parameter("out_partial", [S, D], F32, isOutput=True)

    scale = 1.0 / np.sqrt(DEP)

    with TileContext(nc) as tc:
        with tc.tile_pool(name="singles", bufs=1) as singles:
            # persistent SBUF tensors
            qhT = singles.tile([D2, S], F16, tag="qhT")     # [128, 2048]
            khT = singles.tile([D2, S], F16, tag="khT")
            vh = singles.tile([128, S], F16, tag="vh")     # block st: [128(k),128(d)]
            ctxT = singles.tile([D2, S], F16, tag="ctxT")   # [128(d), 2048(q)] unnormalized
            recip = singles.tile([128, HPC * NT], F32, tag="recip")  # col h*16+qt
            wq_sb = singles.tile([128, 4, D2], F16, tag="wq")
            wk_sb = singles.tile([128, 4, D2], F16, tag="wk")
            wv_sb = singles.tile([128, 4, D2], F16, tag="wv")
            wo_sb = singles.tile([D2, D], F16, tag="wo")
            bq_sb = singles.tile([D2, 1], F32, tag="bq")
            bk_sb = singles.tile([D2, 1], F32, tag="bk")

            nc.sync.dma_start(out=wq_sb[:], in_=wq.ap().rearrange("(c p) d -> p c d", p=128))
            nc.sync.dma_start(out=wk_sb[:], in_=wk.ap().rearrange("(c p) d -> p c d", p=128))
            nc.sync.dma_start(out=wv_sb[:], in_=wv.ap().rearrange("(c p) d -> p c d", p=128))
            nc.sync.dma_start(out=wo_sb[:], in_=wo.ap())
            nc.sync.dma_start(out=bq_sb[:], in_=bq.ap())
            nc.sync.dma_start(out=bk_sb[:], in_=bk.ap())

            # ---------------- stage 1: projections (all fp16 inputs) -------
            with (
                tc.tile_pool(name="xpool", bufs=2) as xpool,
                tc.tile_pool(name="psum1", bufs=2, space="PSUM") as psum1,
            ):
                # v first: vh is not needed until the first AV burst, and
                # q/k gate the main loop.
                x_v = xpool.tile([128, 4, S], F16, tag="x")
                nc.sync.dma_start(
                    out=x_v[:], in_=xvT.ap().rearrange("(c p) s -> p c s", p=128)
                )
                ps = psum1.tile([128, S], F32, tag="ps1")
                for sti in range(NT):
                    for cc in range(4):
                        nc.tensor.matmul(
                            ps[:, sti * 128 : (sti + 1) * 128],
                            x_v[:, cc, sti * 128 : (sti + 1) * 128],
                            wv_sb[:, cc, :],
                            start=(cc == 0),
                            stop=(cc == 3),
                        )
                nc.vector.tensor_copy(out=vh[:], in_=ps[:])
                # qhT = wq.T @ xqT (+bq), khT likewise
                for xT, w_sb, b_sb, outT in (
                    (xqT, wq_sb, bq_sb, qhT),
                    (xkT, wk_sb, bk_sb, khT),
                ):
                    x_sb = xpool.tile([128, 4, S], F16, tag="x")
                    nc.sync.dma_start(
                        out=x_sb[:], in_=xT.ap().rearrange("(c p) s -> p c s", p=128)
                    )
                    ps = psum1.tile([D2, S], F32, tag="ps1")
                    for ncx in range(4):
                        for cc in range(4):
                            nc.tensor.matmul(
                                ps[:, ncx * 512 : (ncx + 1) * 512],
                                w_sb[:, cc, :],
                                x_sb[:, cc, ncx * 512 : (ncx + 1) * 512],
                                start=(cc == 0),
                                stop=(cc == 3),
                            )
                    nc.vector.tensor_scalar_add(outT[:], ps[:], b_sb[:])

            # ---------------- stage 2: attention ----------------
            with (
                tc.tile_pool(name="psum_p", bufs=1, space="PSUM") as pool_p,
                tc.tile_pool(name="psum_pt", bufs=2, space="PSUM") as pool_pt,
                tc.tile_pool(name="Ppool", bufs=3) as Ppool,
                tc.tile_pool(name="PTpool", bufs=2) as PTpool,
                tc.tile_pool(name="sumpool", bufs=4) as sumpool,
            ):
                # Two phases: in phase p, run head hP's P-stream (attn rows,
                # partitions hP*64..) concurrently with head hT=1-hP's
                # PT-stream (transposed probs, the other PE row-group),
                # then a dense AV burst for hT.
                PTs = {}

                def av_chain(hT, qc, c):
                    """8-deep AV accumulation chain (kt = c*8 .. c*8+7)."""
                    ps_av = pool_pt.tile([DEP, 512], F32, tag="pt")
                    PTh = PTs[hT]
                    for j in range(8):
                        kt = c * 8 + j
                        nc.tensor.matmul(
                            ps_av[:],
                            vh[:, kt * 128 + hT * DEP : kt * 128 + (hT + 1) * DEP],
                            PTh[:, kt, qc * 512 : (qc + 1) * 512],
                            start=(j == 0),
                            stop=(j == 7),
                        )
                    return ps_av

                def av_emit(hT, qc):
                    """Both chains for one q-chunk + combine into ctxT."""
                    a0 = av_chain(hT, qc, 0)
                    a1 = av_chain(hT, qc, 1)
                    ct = ctxT[hT * DEP : (hT + 1) * DEP, qc * 512 : (qc + 1) * 512]
                    nc.vector.tensor_copy(out=ct, in_=a0[:])
                    nc.vector.tensor_add(ct, ct, a1[:])

                for phase in range(2):
                    hP, hT = phase, 1 - phase
                    hsP = slice(hP * DEP, (hP + 1) * DEP)
                    hsT = slice(hT * DEP, (hT + 1) * DEP)
                    PTs[hT] = PTpool.tile([128, NT, S], F16, tag="PT", name="PT")
                    for t in range(NT):
                        # ---- PT stream (head hT, k-tile t) ----
                        for half in range(2):
                            ps_t = pool_pt.tile([128, 1024], F32, tag="pt")
                            for ncx in range(2):
                                nc.tensor.matmul(
                                    ps_t[:, ncx * 512 : (ncx + 1) * 512],
                                    khT[hsT, t * 128 : (t + 1) * 128],
                                    qhT[hsT, half * 1024 + ncx * 512 : half * 1024 + (ncx + 1) * 512],
                                    start=True,
                                    stop=True,
                                )
                            nc.scalar.activation(
                                PTs[hT][:, t, half * 1024 : (half + 1) * 1024],
                                ps_t[:],
                                mybir.ActivationFunctionType.Exp,
                                scale=float(scale),
                            )
                        # ---- P stream (head hP, q-tile t), other row-group ----
                        ps_p = pool_p.tile([128, S], F32, tag="pp")
                        for ncx in range(4):
                            nc.tensor.matmul(
                                ps_p[:, ncx * 512 : (ncx + 1) * 512],
                                qhT[hsP, t * 128 : (t + 1) * 128],
                                khT[hsP, ncx * 512 : (ncx + 1) * 512],
                                start=True,
                                stop=True,
                            )
                        P_sb = Ppool.tile([128, S], F32, tag="P")
                        sums = sumpool.tile([128, 1], F32, tag="sums")
                        nc.scalar.activation(
                            P_sb[:],
                            ps_p[:],
                            mybir.ActivationFunctionType.Exp,
                            scale=float(scale),
                            accum_out=sums[:],
                        )
                        rc = recip[:, hP * NT + t : hP * NT + t + 1]
                        nc.vector.reciprocal(rc, sums[:])
                        nc.vector.tensor_scalar_mul(P_sb[:], P_sb[:], rc)
                        nc.sync.dma_start(
                            out=attn_out[hP, t * 128 : (t + 1) * 128, :], in_=P_sb[:]
                        )
                        # ---- previous phase's AV, interleaved for density ----
                        if phase == 1 and t % 4 == 3:
                            av_emit(1, t // 4)

                # ---- tail: AV for head 0 + output projection, overlapped ----
                with tc.tile_pool(name="opool", bufs=3) as opool:
                    ps_o = pool_p.tile([128, S], F32, tag="pp")
                    for t in range(NT):
                        if t % 4 == 0:
                            av_emit(0, t // 4)
                        s1 = ps_o[:, ((2 * t) % 4) * 512 : ((2 * t) % 4 + 1) * 512]
                        s0 = ps_o[:, ((2 * t + 1) % 4) * 512 : ((2 * t + 1) % 4 + 1) * 512]
                        nc.tensor.matmul(
                            s1,
                            ctxT[DEP:, t * 128 : (t + 1) * 128],
                            wo_sb[DEP:, :],
                            start=True,
                            stop=True,
                        )
                        nc.tensor.matmul(
                            s0,
                            ctxT[:DEP, t * 128 : (t + 1) * 128],
                            wo_sb[:DEP, :],
                            start=True,
                            stop=True,
                        )
                        o1 = opool.tile([128, D], F32, tag="o1")
                        nc.scalar.activation(
                            o1[:],
                            s1,
                            mybir.ActivationFunctionType.Copy,
                            scale=recip[:, NT + t : NT + t + 1],
                        )
                        acc = opool.tile([128, D], F32, tag="acc")
                        nc.scalar.activation(
                            acc[:],
                            s0,
                            mybir.ActivationFunctionType.Copy,
                            scale=recip[:, t : t + 1],
                        )
                        nc.vector.tensor_add(acc[:], acc[:], o1[:])
                        nc.sync.dma_start(
                            out=out_partial[t * 128 : (t + 1) * 128, :], in_=acc[:]
                        )

    nc.finalize()
    return nc


def kernel(q, k, v, wq_w, wq_b, wk_w, wk_b, wv_w, wv_b, wo_w, wo_b, _profile=False):
    global _CACHED_NC
    q = np.asarray(q, np.float32)
    k = np.asarray(k, np.float32)
    v = np.asarray(v, np.float32)
    wq_w = np.asarray(wq_w, np.float32)
    wk_w = np.asarray(wk_w, np.float32)
    wv_w = np.asarray(wv_w, np.float32)
    wo_w = np.asarray(wo_w, np.float32)

    if _CACHED_NC is None:
        _CACHED_NC = _build_nc()
    nc = _CACHED_NC

    xT = {}
    for b in range(B):
        xT[("q", b)] = np.ascontiguousarray(q[b].T.astype(np.float16))
        xT[("k", b)] = np.ascontiguousarray(k[b].T.astype(np.float16))
        xT[("v", b)] = np.ascontiguousarray(v[b].T.astype(np.float16))

    in_maps = []
    for c in range(N_CORES):
        b, hp = divmod(c, 4)
        cs = slice(hp * D2, (hp + 1) * D2)
        in_maps.append(
            {
                "xqT": xT[("q", b)],
                "xkT": xT[("k", b)],
                "xvT": xT[("v", b)],
                "wq": np.ascontiguousarray(wq_w[:, cs].astype(np.float16)),
                "wk": np.ascontiguousarray(wk_w[:, cs].astype(np.float16)),
                "wv": np.ascontiguousarray(wv_w[:, cs].astype(np.float16)),
                "wo": np.ascontiguousarray(wo_w[cs, :].astype(np.float16)),
                "bq": np.ascontiguousarray(np.asarray(wq_b, np.float32)[cs, None]),
                "bk": np.ascontiguousarray(np.asarray(wk_b, np.float32)[cs, None]),
            }
        )

    kwargs = {}
    if _profile:
        import os

        os.makedirs("/tmp/bass_trace", exist_ok=True)
        kwargs = {"trace": True, "tmpdir": "/tmp/bass_trace"}
    res = run_bass_kernel_spmd(nc, in_maps, list(range(N_CORES)), **kwargs)

    attn = np.empty((B, H, S, S), np.float32)
    out = np.zeros((B, S, D), np.float32)
    for c in range(N_CORES):
        b, hp = divmod(c, 4)
        attn[b, 2 * hp : 2 * hp + 2] = res.results[c]["attn_out"]
        out[b] += res.results[c]["out_partial"]
    # fold v/o biases: softmax rows sum to 1 -> ctx += wv_b, out += wv_b@wo + wo_b
    out += (
        np.asarray(wv_b, np.float32) @ wo_w + np.asarray(wo_b, np.float32)
    )[None, None, :]

    if _profile:
        return (out, attn), res
    return out, attn
